# revision 1
# baseline (speedup 1.0000x reference)
"""Trainium2 Bass kernel for nn_FFMLP (4-layer MLP, hidden=128, relu).

Strategy (pure data parallel, batch sharded 8 ways):
- Feature-major on-chip layout: activations live as [feat, batch] so every
  layer is a single K<=128 matmul with the (tiny, replicated) weight as the
  stationary operand and the activation stream as the moving operand.
- fp16 matmul operands (1 cycle/row on the PE vs 4 for fp32), fp32 PSUM
  accumulation; host rounds inputs/weights to fp16 (unbiased ~2^-11).
- Per-512-column chunk pipeline: L0..L3 outputs each occupy one PSUM bank;
  ReLU + downcast PSUM->SBUF is split between ScalarE and VectorE (the
  structural bottleneck: ~1 elem/cycle/lane each from PSUM).
- L4 (M=16) is packed 4 chunks deep into one PSUM bank via column tiling
  (tile_position=(0,32j)) so the final fp32 evacuation is amortized 4x.
- Host transposes x -> x.T per shard and the [16, B/8] result back.
"""
import sys

if "/opt/trn_rl_repo" not in sys.path:
    sys.path.insert(0, "/opt/trn_rl_repo")

import numpy as np

import concourse.bass as bass
import concourse.mybir as mybir
import concourse.tile as tile

INPUT_DIM = 32
OUTPUT_DIM = 16
HIDDEN = 128
PADDED_OUT = 16
NUM_LAYERS = 4
B = 524288
N_CORES = 8
B_CORE = B // N_CORES  # 65536
CHUNK = 512
N_CHUNKS = B_CORE // CHUNK  # 128
GROUP = 4  # chunks packed per L4 PSUM bank (column tiling)
IN_SLAB = 8  # chunks per input DMA

fp16 = mybir.dt.float16
fp32 = mybir.dt.float32
RELU = mybir.ActivationFunctionType.Relu


def _split_waits(nc, max_waits=1):
    """walrus in this image rejects >1 semaphore wait per instruction on some
    formats; split excess waits onto preceding NOPs on the same engine queue
    (queues are in-order, so semantics are preserved)."""
    n_new = 0
    for bb in nc.main_func.blocks:
        out_list = []
        changed = False
        for ins in bb.instructions:
            si = ins.sync_info
            if si is not None and si.on_wait and len(si.on_wait) > max_waits:
                waits = list(si.on_wait)
                extra, keep = waits[:-max_waits], waits[-max_waits:]
                while extra:
                    chunk, extra = extra[:max_waits], extra[max_waits:]
                    n_new += 1
                    nop = mybir.InstNoOp(name=f"I-waitsplit-{n_new}", ins=[], outs=[])
                    nop.engine = ins.engine
                    nop.sync_info = mybir.SyncInfo(on_wait=chunk, on_update=[])
                    out_list.append(nop)
                ins.sync_info = mybir.SyncInfo(on_wait=keep, on_update=si.on_update)
                changed = True
            out_list.append(ins)
        if changed:
            bb.instructions = out_list
    return n_new


def build(n_chunks=N_CHUNKS):
    nc = bass.Bass()
    n_cols = n_chunks * CHUNK
    xt = nc.declare_dram_parameter("xt", [INPUT_DIM, n_cols], fp16, isOutput=False)
    w0 = nc.declare_dram_parameter("w0", [INPUT_DIM, HIDDEN], fp16, isOutput=False)
    w1 = nc.declare_dram_parameter("w1", [HIDDEN, HIDDEN], fp16, isOutput=False)
    w2 = nc.declare_dram_parameter("w2", [HIDDEN, HIDDEN], fp16, isOutput=False)
    w3 = nc.declare_dram_parameter("w3", [HIDDEN, HIDDEN], fp16, isOutput=False)
    w4 = nc.declare_dram_parameter("w4", [HIDDEN, PADDED_OUT], fp16, isOutput=False)
    yt = nc.declare_dram_parameter("yt", [PADDED_OUT, n_cols], fp32, isOutput=True)

    with tile.TileContext(nc) as tc:
        with (
            tc.tile_pool(name="wp", bufs=1) as wp,
            tc.tile_pool(name="io", bufs=1) as io,
            tc.tile_pool(name="hp", bufs=1) as hp,
            tc.tile_pool(name="ps", bufs=1, space="PSUM") as ps,
        ):
            w0s = wp.tile([INPUT_DIM, HIDDEN], fp16, tag="w0", name="w0s")
            w1s = wp.tile([HIDDEN, HIDDEN], fp16, tag="w1", name="w1s")
            w2s = wp.tile([HIDDEN, HIDDEN], fp16, tag="w2", name="w2s")
            w3s = wp.tile([HIDDEN, HIDDEN], fp16, tag="w3", name="w3s")
            w4s = wp.tile([HIDDEN, PADDED_OUT], fp16, tag="w4", name="w4s")
            nc.sync.dma_start(out=w0s, in_=w0[:, :])
            nc.sync.dma_start(out=w1s, in_=w1[:, :])
            nc.sync.dma_start(out=w2s, in_=w2[:, :])
            nc.sync.dma_start(out=w3s, in_=w3[:, :])
            nc.sync.dma_start(out=w4s, in_=w4[:, :])

            xslab = None
            p4 = None
            for c in range(n_chunks):
                if c % IN_SLAB == 0:
                    nslab = min(IN_SLAB, n_chunks - c)
                    xslab = io.tile(
                        [INPUT_DIM, nslab * CHUNK], fp16,
                        tag="xin", bufs=3, name="xslab",
                    )
                    nc.sync.dma_start(
                        out=xslab,
                        in_=xt[:, c * CHUNK : (c + nslab) * CHUNK],
                    )
                o = (c % IN_SLAB) * CHUNK
                xa = xslab[:, o : o + CHUNK]
                j = c % GROUP

                p0 = ps.tile([HIDDEN, CHUNK], fp32, tag="l0", bufs=2, name="p0")
                nc.tensor.matmul(p0[:, :], w0s[:, :], xa, start=True, stop=True)
                h1 = hp.tile([HIDDEN, CHUNK], fp16, tag="h1", bufs=2, name="h1")
                nc.scalar.activation(h1[:, :], p0[:, :], RELU)

                p1 = ps.tile([HIDDEN, CHUNK], fp32, tag="l1", bufs=2, name="p1")
                nc.tensor.matmul(p1[:, :], w1s[:, :], h1[:, :], start=True, stop=True)
                h2 = hp.tile([HIDDEN, CHUNK], fp16, tag="h2", bufs=2, name="h2")
                nc.vector.tensor_scalar_max(h2[:, :], p1[:, :], 0.0)

                p2 = ps.tile([HIDDEN, CHUNK], fp32, tag="l2", bufs=1, name="p2")
                nc.tensor.matmul(p2[:, :], w2s[:, :], h2[:, :], start=True, stop=True)
                h3 = hp.tile([HIDDEN, CHUNK], fp16, tag="h3", bufs=2, name="h3")
                nc.scalar.activation(h3[:, :], p2[:, :], RELU)

                p3 = ps.tile([HIDDEN, CHUNK], fp32, tag="l3", bufs=2, name="p3")
                nc.tensor.matmul(p3[:, :], w3s[:, :], h3[:, :], start=True, stop=True)
                h4 = hp.tile([HIDDEN, CHUNK], fp16, tag="h4", bufs=2, name="h4")
                nc.vector.tensor_scalar_max(h4[:, :], p3[:, :], 0.0)

                if j == 0:
                    p4 = ps.tile([HIDDEN, CHUNK], fp32, tag="l4", bufs=1, name="p4")
                nc.tensor.matmul(
                    p4[32 * j : 32 * j + PADDED_OUT, :],
                    w4s[:, :],
                    h4[:, :],
                    start=True,
                    stop=True,
                    tile_position=(0, 32 * j),
                )

                if j == GROUP - 1 or c == n_chunks - 1:
                    osb = io.tile([HIDDEN, CHUNK], fp32, tag="osb", bufs=2, name="osb")
                    nc.scalar.copy(out=osb[:, :], in_=p4[:, :])
                    g0 = (c // GROUP) * GROUP
                    for jj in range(j + 1):
                        nc.sync.dma_start(
                            out=yt[:, (g0 + jj) * CHUNK : (g0 + jj + 1) * CHUNK],
                            in_=osb[32 * jj : 32 * jj + PADDED_OUT, :],
                        )
    _split_waits(nc)
    return nc


def _split_weights(weights):
    ws = []
    off = 0
    ws.append(weights[off : off + HIDDEN * INPUT_DIM].reshape(HIDDEN, INPUT_DIM))
    off += HIDDEN * INPUT_DIM
    for _ in range(NUM_LAYERS - 1):
        ws.append(weights[off : off + HIDDEN * HIDDEN].reshape(HIDDEN, HIDDEN))
        off += HIDDEN * HIDDEN
    ws.append(weights[off : off + PADDED_OUT * HIDDEN].reshape(PADDED_OUT, HIDDEN))
    return ws


_NC_CACHE = {}


def kernel(inputs: np.ndarray, weights: np.ndarray) -> np.ndarray:
    from concourse.bass_utils import run_bass_kernel_spmd

    assert inputs.shape == (B, INPUT_DIM), inputs.shape
    ws = _split_weights(np.asarray(weights, dtype=np.float32))
    # stationary operands are lhsT = [K_in, M_out] = W.T
    wmaps = {
        "w0": np.ascontiguousarray(ws[0].T).astype(np.float16),
        "w1": np.ascontiguousarray(ws[1].T).astype(np.float16),
        "w2": np.ascontiguousarray(ws[2].T).astype(np.float16),
        "w3": np.ascontiguousarray(ws[3].T).astype(np.float16),
        "w4": np.ascontiguousarray(ws[4].T).astype(np.float16),
    }
    in_maps = []
    for i in range(N_CORES):
        xc = inputs[i * B_CORE : (i + 1) * B_CORE]
        xtc = np.ascontiguousarray(xc.T).astype(np.float16)
        in_maps.append({"xt": xtc, **wmaps})

    if "nc" not in _NC_CACHE:
        _NC_CACHE["nc"] = build()
    nc = _NC_CACHE["nc"]
    res = run_bass_kernel_spmd(nc, in_maps, list(range(N_CORES)))
    outs = [np.ascontiguousarray(r["yt"].T) for r in res.results]
    return np.concatenate(outs, axis=0)[:, :OUTPUT_DIM]
